# revision 16
# baseline (speedup 1.0000x reference)
"""Trainium2 Bass kernel for nn_DeepSupervisionBoundaryDoULoss.

kernel(**inputs) takes the FULL unsharded inputs (logits0/1/2, targets,
valid_mask) and returns the full scalar loss (float32).

Strategy: data-parallel over the 32 (b,n) pairs -> 4 pairs per core x 8 cores.
Each core streams its slice (~9.25 MiB) once; DMA is the roofline (~29us at
~340 GB/s/core). Pair-major pipeline; every queue carries the data in the
order compute consumes it (targets on the gpsimd SWDGE queue, logits0 on the
sync HWDGE queue, logits1+2 on the scalar HWDGE queue).

  - Pool(gpsimd): int32->fp8e4 casting DMAs for targets (deinterleaved rows
    A/B/C/D as [128, half, parity, 516] w/ 2-col zero pads), pad memsets.
  - ACT(scalar):  logits1/2 DMA issue (in the dead preamble window); sigmoid
    (f32->bf16, one op for merged s1+s2); z = sum(p^2) via Square w/ accum.
  - DVE(vector):  inter = sum(p*t) via scalar_tensor_tensor (bf16 x fp8);
    interior counts via stt-relu ((nsum-4) max 0) straight from PSUM with
    accum_out (one op over the 4-bank s0 PSUM + one merged s1+s2 op);
    t2 extraction copy.
  - PE(tensor):   3x3-cross conv as fp8 DoubleRow matmuls: per tile ONE DR
    fuses [I|band]@[center|other-parity] and ONE DR fuses [I|I]@[left|right]
    (overlapping strided ifmap APs), accumulated in PSUM f32.
  - Sync(SP):     consts + logits0 DMAs + per-pair stats out-DMAs.

All engine scratch is per-op-tagged so ACT and DVE never couple through
buffer reuse. Stats land in [128,10] f32 blocks per pair (partition-reduced
on host); the host finishes alpha/dou/weighted mean plus a seam correction
for the 4 rows/pair the on-chip conv cannot see (rows 255/256 at s0,
127/128 at s1), and splits the merged s1+s2 count using the (targets-only)
host s2 interior count.
"""

from contextlib import ExitStack

import numpy as np

N_PAIRS = 4
N_CORES = 8
H0, H1, H2 = 512, 256, 128
N_SCALES = 3
SMOOTH = 1e-5

# per-pair stats columns (s0 quantities split per half)
C_CNT0A, C_CNT0B, C_CNT1 = 0, 1, 2
C_INT0A, C_INT0B, C_INT1, C_INT2 = 3, 4, 5, 6
C_Z0A, C_Z0B, C_Z1, C_Z2 = 7, 8, 9, 10
COLS_PER_PAIR = 12

# consts layout (fp8), free-dim offsets
W_IB2M = 0      # [2,128]  DR weights [I | B2M]
W_B2PI = 256    # [2,128]  [B2P | I]
W_II = 512      # [2,128]  [I | I]
W_ITRI = 768    # [2,128]  [I | TRI]
W_I = 1024      # [128]    plain identity
W_EVEN = 1152   # [64]     even-partition selector
N_CONST = 1216

_NC_CACHE = {}


def make_consts():
    import ml_dtypes

    ident = np.eye(128, dtype=np.float32)
    b2m = np.zeros((128, 128), np.float32)  # q in {i-1, i}
    b2p = np.zeros((128, 128), np.float32)  # q in {i, i+1}
    tri = np.zeros((128, 128), np.float32)  # q in {i-1, i, i+1}
    for i in range(128):
        for dq in (-1, 0):
            if 0 <= i + dq < 128:
                b2m[i + dq, i] = 1.0
        for dq in (0, 1):
            if 0 <= i + dq < 128:
                b2p[i + dq, i] = 1.0
        for dq in (-1, 0, 1):
            if 0 <= i + dq < 128:
                tri[i + dq, i] = 1.0
    even = np.zeros((128, 64), np.float32)
    for i in range(64):
        even[2 * i, i] = 1.0
    c = np.zeros((128, N_CONST), np.float32)
    c[:, 0:128], c[:, 128:256] = ident, b2m
    c[:, 256:384], c[:, 384:512] = b2p, ident
    c[:, 512:640], c[:, 640:768] = ident, ident
    c[:, 768:896], c[:, 896:1024] = ident, tri
    c[:, 1024:1152] = ident
    c[:, 1152:1216] = even
    return c.astype(ml_dtypes.float8_e4m3fn)


def build_kernel(n_pairs=N_PAIRS):
    import concourse.tile as tile
    from concourse import bacc, mybir
    from bass_rust import AP

    F32 = mybir.dt.float32
    F8 = mybir.dt.float8e4
    BF16 = mybir.dt.bfloat16
    I32 = mybir.dt.int32
    ALU = mybir.AluOpType
    ACTF = mybir.ActivationFunctionType
    DRM = mybir.MatmulPerfMode.DoubleRow

    ncols = n_pairs * COLS_PER_PAIR
    nc = bacc.Bacc("TRN2", target_bir_lowering=False, debug=False)

    logits0 = nc.dram_tensor("logits0", [n_pairs, H0, H0], F32, kind="ExternalInput").ap()
    logits1 = nc.dram_tensor("logits1", [n_pairs, H1, H1], F32, kind="ExternalInput").ap()
    logits2 = nc.dram_tensor("logits2", [n_pairs, H2, H2], F32, kind="ExternalInput").ap()
    targets = nc.dram_tensor("targets", [n_pairs, H0, H0], I32, kind="ExternalInput").ap()
    consts8 = nc.dram_tensor("consts_f8", [128, N_CONST], F8, kind="ExternalInput").ap()
    out = nc.dram_tensor("out", [128, ncols], F32, kind="ExternalOutput").ap()

    with tile.TileContext(nc) as tc, ExitStack() as ctx:
        singles = ctx.enter_context(tc.tile_pool(name="singles", bufs=1))
        tpool = ctx.enter_context(tc.tile_pool(name="tpool", bufs=4))
        lpool = ctx.enter_context(tc.tile_pool(name="lpool", bufs=4))
        ppool = ctx.enter_context(tc.tile_pool(name="ppool", bufs=4))
        spool = ctx.enter_context(tc.tile_pool(name="spool", bufs=2))
        ps0p = ctx.enter_context(tc.tile_pool(name="ps0p", bufs=2, space="PSUM"))
        ps12p = ctx.enter_context(tc.tile_pool(name="ps12p", bufs=2, space="PSUM"))

        cb = singles.tile([128, N_CONST], F8)
        nc.sync.dma_start(out=cb, in_=consts8)
        stats = singles.tile([128, ncols], F32)
        nc.vector.memset(stats, 0.0)
        zeros1 = singles.tile([128, 1], BF16)
        nc.vector.memset(zeros1, 0.0)

        def wdr(off):
            return cb[:, off : off + 256].rearrange("p (two m) -> p two m", two=2)

        def windows(t, off, bstride, nb, istride, n):
            pstride = 1
            for s in t.tensor.shape[1:]:
                pstride *= s
            return AP(tensor=t.tensor, offset=off,
                      ap=[[pstride, 128], [bstride, nb], [istride, n]])

        # ---- all input DMAs queued up front ----
        # Spread the big logits0 stream across BOTH hwdge queues (half 0 on
        # sync/Q1, half 1 on scalar/Q10) so each pair's 1 MiB arrives at
        # 2-queue bandwidth; targets + logits1/2 ride the gpsimd SWDGE queue.
        # A dummy activation first forces the ACT function-table load into
        # the dead preamble window instead of in front of the first sigmoid.
        dummy = singles.tile([128, 1], BF16)
        nc.scalar.activation(out=dummy, in_=zeros1, func=ACTF.Sigmoid)
        l0s = []
        for p in range(n_pairs):
            l0 = lpool.tile([128, 2, 2, 512], F32, tag="l0", name=f"l0_{p}")
            nc.sync.dma_start(
                out=l0[:, 0],
                in_=logits0[p, 0:256].rearrange("(r parity) c -> r parity c", parity=2),
            )
            nc.scalar.dma_start(
                out=l0[:, 1],
                in_=logits0[p, 256:512].rearrange("(r parity) c -> r parity c", parity=2),
            )
            l0s.append(l0)
        t0s, l12s = [], []
        for p in range(n_pairs):
            t0 = tpool.tile([128, 2, 2, 516], F8, tag="t0", name=f"t0_{p}")
            for half in range(2):
                nc.gpsimd.dma_start(
                    out=t0[:, half, :, 2:514],
                    in_=targets[p, half * 256 : (half + 1) * 256].rearrange(
                        "(r parity) c -> r parity c", parity=2
                    ),
                )
            nc.gpsimd.memset(t0[:, :, :, 0:2], 0.0)
            nc.gpsimd.memset(t0[:, :, :, 514:516], 0.0)
            t0s.append(t0)
            l12 = lpool.tile([128, 640], F32, tag="l12", name=f"l12_{p}")
            nc.gpsimd.dma_start(
                out=l12[:, 0:512].rearrange("r (g c) -> r g c", g=2),
                in_=logits1[p].rearrange("(g r) c -> r g c", g=2),
            )
            nc.gpsimd.dma_start(out=l12[:, 512:640], in_=logits2[p])
            l12s.append(l12)

        stcol = lambda p, c: stats[:, p * COLS_PER_PAIR + c : p * COLS_PER_PAIR + c + 1]

        p0s, p12s = [], []

        def emit_main(p):
            """Sigmoids + inter + conv + counts for pair p (no z-squares)."""
            t0, l0, l12 = t0s[p], l0s[p], l12s[p]

            # ---------------- scale 0 ----------------
            p0 = ppool.tile([128, 2, 2, 512], BF16, tag="p0", name=f"p0_{p}")
            p0s.append(p0)
            for half in range(2):
                nc.scalar.activation(out=p0[:, half], in_=l0[:, half],
                                     func=ACTF.Sigmoid)
            int0 = spool.tile([128, 2, 2, 512], BF16, tag="int0", name=f"int0_{p}")
            nc.vector.scalar_tensor_tensor(
                out=int0, in0=p0, scalar=1.0, in1=t0[:, :, :, 2:514],
                op0=ALU.mult, op1=ALU.mult, accum_out=stcol(p, C_INT0A),
            )
            for half in range(2):
                ps0 = ps0p.tile([128, 2, 512], F32, tag="ps0",
                                name=f"ps0_{p}_{half}")
                for parity in range(2):
                    dst = ps0[:, parity, :]
                    w1 = wdr(W_IB2M if parity == 0 else W_B2PI)
                    nc.tensor.matmul(dst, w1, t0[:, half, :, 2:514],
                                     start=True, stop=False, perf_mode=DRM)
                    off = (half * 2 + parity) * 516 + 1
                    nc.tensor.matmul(dst, wdr(W_II), windows(t0, off, 2, 2, 1, 512),
                                     start=False, stop=True, perf_mode=DRM)
                cnt0 = spool.tile([128, 2, 512], BF16, tag=f"cnt0{half}",
                                  name=f"cnt0_{p}_{half}")
                nc.vector.scalar_tensor_tensor(
                    out=cnt0, in0=ps0, scalar=-4.0,
                    in1=zeros1.broadcast_to([128, 2, 512]),
                    op0=ALU.add, op1=ALU.max, accum_out=stcol(p, C_CNT0A + half),
                )

            # ---------------- scales 1+2 ----------------
            p12 = ppool.tile([128, 640], BF16, tag="p12", name=f"p12_{p}")
            p12s.append(p12)
            nc.scalar.activation(out=p12, in_=l12, func=ACTF.Sigmoid)
            int1 = spool.tile([128, 2, 256], BF16, tag="int1", name=f"int1_{p}")
            nc.vector.scalar_tensor_tensor(
                out=int1, in0=p12[:, 0:512].rearrange("r (g c) -> r g c", g=2),
                scalar=1.0, in1=t0[:, :, 0, 2:514:2],
                op0=ALU.mult, op1=ALU.mult, accum_out=stcol(p, C_INT1),
            )
            # ps12: [0:512]=s1 conv, [512:640]=t2 extract. (The s2 interior
            # count is recovered on the host from targets, which it needs
            # anyway -- no on-device s2 conv.)
            ps12 = ps12p.tile([128, 640], F32, tag="ps12", name=f"ps12_{p}")
            for g in range(2):
                dst = ps12[:, g * 256 : (g + 1) * 256]
                off = (g * 2) * 516
                nc.tensor.matmul(dst, wdr(W_ITRI), windows(t0, off, 2, 2, 2, 256),
                                 start=True, stop=False, perf_mode=DRM)
                nc.tensor.matmul(dst, cb[:, W_I : W_I + 128],
                                 t0[:, g, 0, 4:516:2], start=False, stop=True)
            for g in range(2):
                nc.tensor.matmul(ps12[g * 64 : (g + 1) * 64, 512:640],
                                 cb[:, W_EVEN : W_EVEN + 64],
                                 t0[:, g, 0, 2:514:4], start=True, stop=True)
            t2 = tpool.tile([128, 128], F8, tag="t2", name=f"t2_{p}")
            nc.vector.tensor_copy(t2, ps12[:, 512:640])
            int2 = spool.tile([128, 128], BF16, tag="int2", name=f"int2_{p}")
            nc.vector.scalar_tensor_tensor(
                out=int2, in0=p12[:, 512:640], scalar=1.0, in1=t2,
                op0=ALU.mult, op1=ALU.mult, accum_out=stcol(p, C_INT2),
            )
            cnt1 = spool.tile([128, 512], BF16, tag="cnt1", name=f"cnt1_{p}")
            nc.vector.scalar_tensor_tensor(
                out=cnt1, in0=ps12[:, 0:512],
                scalar=-4.0, in1=zeros1.broadcast_to([128, 512]),
                op0=ALU.add, op1=ALU.max, accum_out=stcol(p, C_CNT1),
            )

        def emit_z(p):
            """z = sum(p^2) squares (deferred one pair) + stats out-DMA."""
            p0, p12 = p0s[p], p12s[p]
            sqr0 = spool.tile([128, 2, 2, 512], BF16, tag="sqr0", name=f"sqr0_{p}")
            nc.scalar.activation(out=sqr0, in_=p0, func=ACTF.Square,
                                 accum_out=stcol(p, C_Z0A))
            nc.scalar.activation(
                out=spool.tile([128, 512], BF16, tag="sqr1", name=f"sqr1_{p}"),
                in_=p12[:, 0:512], func=ACTF.Square, accum_out=stcol(p, C_Z1))
            nc.scalar.activation(
                out=spool.tile([128, 128], BF16, tag="sqr2", name=f"sqr2_{p}"),
                in_=p12[:, 512:640], func=ACTF.Square, accum_out=stcol(p, C_Z2))
            nc.sync.dma_start(
                out=out[:, p * COLS_PER_PAIR : p * COLS_PER_PAIR + 11],
                in_=stats[:, p * COLS_PER_PAIR : p * COLS_PER_PAIR + 11],
            )

        emit_main(0)
        emit_main(1)
        emit_z(0)
        emit_main(2)
        emit_z(1)
        emit_main(3)
        emit_z(2)
        emit_z(3)

    nc.compile()
    return nc


def get_kernel():
    if "nc" not in _NC_CACHE:
        _NC_CACHE["nc"] = build_kernel(N_PAIRS)
    return _NC_CACHE["nc"]


def seam_interior_counts(tg_pair):
    """Interior pixels in the seam rows the kernel cannot see (per scale)."""
    tg_pair = np.asarray(tg_pair)
    out = []
    for h in (H0, H1):
        step = H0 // h
        t = tg_pair[::step, ::step].astype(np.float64)
        pad = np.pad(t, 1)
        cnt = 0
        for r in (h // 2 - 1, h // 2):
            pr = r + 1
            nsum = (
                pad[pr, 1:-1] + pad[pr - 1, 1:-1] + pad[pr + 1, 1:-1]
                + pad[pr, 0:-2] + pad[pr, 2:]
            )
            cnt += int((nsum == 5.0).sum())
        out.append(float(cnt))
    out.append(0.0)
    return out


def combine_stats(all_core_outs, valid_mask, targets, n_pairs=N_PAIRS):
    vm = (np.asarray(valid_mask, np.float32).reshape(-1) >= 0.5).astype(np.float64)
    tg = np.asarray(targets).reshape(-1, H0, H0)
    n_total = vm.shape[0]
    per = np.zeros((N_SCALES, n_total), np.float64)
    for core, st in enumerate(all_core_outs):
        st = np.asarray(st, np.float64).sum(axis=0)  # reduce 128 partitions
        for j in range(n_pairs):
            g = core * n_pairs + j
            c = st[j * COLS_PER_PAIR : (j + 1) * COLS_PER_PAIR]
            seam = seam_interior_counts(tg[g])
            tgg = tg[g].astype(np.float64)
            host_S = [tgg.sum(), tgg[::2, ::2].sum(), tgg[::4, ::4].sum()]
            # s2 interior fully host-side (targets-only bookkeeping)
            t2 = tgg[::4, ::4]
            pad = np.pad(t2, 1)
            nsum2 = (pad[1:-1, 1:-1] + pad[:-2, 1:-1] + pad[2:, 1:-1]
                     + pad[1:-1, :-2] + pad[1:-1, 2:])
            int2_host = float(((nsum2 == 5.0) & (t2 == 1.0)).sum())
            interior = [c[C_CNT0A] + c[C_CNT0B], c[C_CNT1], int2_host]
            inter_v = [c[C_INT0A] + c[C_INT0B], c[C_INT1], c[C_INT2]]
            z_v = [c[C_Z0A] + c[C_Z0B], c[C_Z1], c[C_Z2]]
            for s in range(N_SCALES):
                S = host_S[s]
                C = S - (interior[s] + seam[s])
                alpha = min(2.0 * (1.0 - (C + SMOOTH) / (S + SMOOTH)) - 1.0, 0.8)
                dou = (z_v[s] + S - 2.0 * inter_v[s] + SMOOTH) / (
                    z_v[s] + S - (1.0 + alpha) * inter_v[s] + SMOOTH
                )
                per[s, g] = dou if S > 0 else 0.0
    cnt = vm.sum()
    ws = np.array([1.0, 0.5, 0.25])
    ws = ws / ws.sum()
    loss = 0.0
    for s in range(N_SCALES):
        ls = (per[s] * vm).sum() / cnt if cnt > 0 else 0.0
        loss += ws[s] * ls
    return np.float32(loss)


def make_in_maps(inputs):
    l0 = np.ascontiguousarray(np.asarray(inputs["logits0"], np.float32).reshape(-1, H0, H0))
    l1 = np.ascontiguousarray(np.asarray(inputs["logits1"], np.float32).reshape(-1, H1, H1))
    l2 = np.ascontiguousarray(np.asarray(inputs["logits2"], np.float32).reshape(-1, H2, H2))
    tg = np.ascontiguousarray(np.asarray(inputs["targets"], np.int32).reshape(-1, H0, H0))
    consts = np.asarray(make_consts())
    in_maps = []
    for core in range(N_CORES):
        lo, hi = core * N_PAIRS, (core + 1) * N_PAIRS
        in_maps.append({
            "logits0": np.ascontiguousarray(l0[lo:hi]),
            "logits1": np.ascontiguousarray(l1[lo:hi]),
            "logits2": np.ascontiguousarray(l2[lo:hi]),
            "targets": np.ascontiguousarray(tg[lo:hi]),
            "consts_f8": consts,
        })
    return in_maps


def run_cores(inputs, **spmd_kwargs):
    from concourse.bass_utils import run_bass_kernel_spmd

    nc = get_kernel()
    in_maps = make_in_maps(inputs)
    return run_bass_kernel_spmd(nc, in_maps, core_ids=list(range(N_CORES)), **spmd_kwargs)


def kernel(**inputs) -> np.ndarray:
    res = run_cores(inputs)
    outs = [res.results[c]["out"] for c in range(N_CORES)]
    return combine_stats(outs, inputs["valid_mask"], inputs["targets"])
